# revision 1
# baseline (speedup 1.0000x reference)
"""Trainium2 Bass kernel for nn_DiseaseHead.

Computation (per the reference):
    w_rs = weights.sum(-1)                       # [P, C]
    pw   = features * w_rs + bias                # [B, P, C]
    pwn  = LayerNorm_c(pw) * gamma + beta
    h    = relu(features + pwn)
    out  = einsum("bpc,dc->bpd", h, lin_w) + lin_b

Sharding: data-parallel over batch B across 8 NeuronCores; parameters
replicated.  All bulk compute in bf16 (tolerance 2e-2).

Per-core structure, per group (128 batch rows x PG=5 points x C=256):
 - f loaded [128b, 5p, 256c] bf16; fT loaded host-pretransposed
   [c, b] bf16 strips per point-group.
 - LN stats via "flip" matmuls: fT / f2T (=fT^2, squared on Pool) as the
   STATIONARY operand, tiny per-point weight columns moving =>
   mu = sum(w_rs*f + bias)/C and E2 = E[pw^2] land directly in [128b, pt]
   partition form (no transpose-back).  sb/sb2 constants enter via a
   ones-stationary rank-1 accumulate.
 - smalls: var = E2 - mu^2, r = 1/sqrt(var+eps) (DVE+ACT), m2 = -r*mu.
 - q = pw1*r + m2 per point on DVE tensor_scalar (4x bf16 mode), where
   pw1 = f*w_rs (DVE TT).  The bias term enters transposed-space as a
   masked rank-5 matmul bias^T (x) r.
 - zT = T(q) + T(f) + bias^T(x)r accumulated in f32 PSUM via regular
   bf16 matmuls (identity moving); ACT drains with fused Relu -> hT bf16.
 - proj: hT as stationary, lin_w^T moving => out[128b, pt*5d] in PSUM;
   ACT copies out; DMA stores [BS, P, D] contiguous 100B rows.
"""

import sys

if "/opt/trn_rl_repo" not in sys.path:
    sys.path.insert(0, "/opt/trn_rl_repo")

import numpy as np

B, P, C, D = 8192, 25, 256, 5
NCORES = 8
BS = B // NCORES          # 1024 batch rows per core
PG = 5                    # points per group
NPG = P // PG             # 5 point-groups
NBT = BS // 128           # 8 batch tiles of 128 rows
LN_EPS = 1e-5

_cache = {}


def _build(has_gamma: bool, has_beta: bool, has_linb: bool):
    import concourse.bass as bass
    import concourse.tile as tile
    from concourse import bacc, mybir
    from concourse.masks import make_identity

    f32 = mybir.dt.float32
    bf16 = mybir.dt.bfloat16
    Alu = mybir.AluOpType
    Act = mybir.ActivationFunctionType

    nc = bacc.Bacc("TRN2", target_bir_lowering=False, debug=False,
                   num_devices=NCORES)

    x = nc.dram_tensor("x", [P, BS, C], bf16, kind="ExternalInput")
    xt = nc.dram_tensor("xt", [P, C, BS], bf16, kind="ExternalInput")
    wb = nc.dram_tensor("wb", [P * C], bf16, kind="ExternalInput")
    bt5 = nc.dram_tensor("bt5", [5, P, 2, 128], bf16, kind="ExternalInput")
    wmv = nc.dram_tensor("wmv", [128, P, 2, 2], bf16, kind="ExternalInput")
    wmv2 = nc.dram_tensor("wmv2", [128, P, 2], bf16, kind="ExternalInput")
    sbr = nc.dram_tensor("sbr", [P, 2], bf16, kind="ExternalInput")
    lwt = nc.dram_tensor("lwt", [128, 2, D], bf16, kind="ExternalInput")
    g = nc.dram_tensor("g", [C], f32, kind="ExternalInput")
    bt_ = nc.dram_tensor("bt", [C], f32, kind="ExternalInput")
    lb = nc.dram_tensor("lb", [D], f32, kind="ExternalInput")
    y = nc.dram_tensor("y", [BS, P, D], f32, kind="ExternalOutput")

    with tile.TileContext(nc) as tc:
        with (
            tc.tile_pool(name="consts", bufs=1) as consts,
            tc.tile_pool(name="ftp", bufs=3) as ftp,
            tc.tile_pool(name="f2tp", bufs=3) as f2tp,
            tc.tile_pool(name="fp", bufs=5) as fp,
            tc.tile_pool(name="pw1p", bufs=4) as pw1p,
            tc.tile_pool(name="qp", bufs=4) as qp,
            tc.tile_pool(name="htp", bufs=3) as htp,
            tc.tile_pool(name="statp", bufs=4) as statp,
            tc.tile_pool(name="smallp", bufs=16) as smallp,
            tc.tile_pool(name="rrowp", bufs=4) as rrowp,
            tc.tile_pool(name="op", bufs=4) as op_,
            tc.tile_pool(name="zt", bufs=2, space="PSUM") as ztp,
            tc.tile_pool(name="smps", bufs=2, space="PSUM") as smps,
        ):
            ident = consts.tile([128, 128], f32)
            make_identity(nc, ident[:])
            i16 = consts.tile([128, 128], bf16)
            nc.vector.tensor_copy(i16[:], ident[:])

            eps_t = consts.tile([128, 1], f32)
            nc.vector.memset(eps_t[:], LN_EPS)

            wB = consts.tile([128, P, C], bf16)
            nc.sync.dma_start(
                out=wB[:], in_=bass.AP(wb, 0, [[0, 128], [1, P * C]]))
            bT5 = consts.tile([5, P, 2, 128], bf16)
            nc.sync.dma_start(out=bT5[:], in_=bt5[:])
            wMov = consts.tile([128, P, 2, 2], bf16)
            nc.sync.dma_start(out=wMov[:], in_=wmv[:])
            wMov2 = consts.tile([128, P, 2], bf16)
            nc.sync.dma_start(out=wMov2[:], in_=wmv2[:])
            sbRow = consts.tile([1, P, 2], bf16)
            nc.sync.dma_start(
                out=sbRow[:], in_=bass.AP(sbr, 0, [[0, 1], [2, P], [1, 2]]))
            ones1 = consts.tile([1, 128], bf16)
            nc.vector.memset(ones1[:], 1.0)
            lwT = consts.tile([128, 2, D], bf16)
            nc.sync.dma_start(out=lwT[:], in_=lwt[:])

            if has_gamma:
                gB = consts.tile([128, C], bf16)
                nc.gpsimd.dma_start(
                    out=gB[:], in_=bass.AP(g, 0, [[0, 128], [1, C]]))
            if has_beta:
                bB2 = consts.tile([128, C], bf16)
                nc.gpsimd.dma_start(
                    out=bB2[:], in_=bass.AP(bt_, 0, [[0, 128], [1, C]]))
            if has_linb:
                lbB = consts.tile([128, PG, D], f32)
                nc.gpsimd.dma_start(
                    out=lbB[:], in_=bass.AP(lb, 0, [[0, 128], [0, PG], [1, D]]))

            pg_tiles = {}

            def load_pg(pg):
                if pg in pg_tiles or pg >= NPG:
                    return
                ft = ftp.tile([128, PG, 2, 1024], bf16, tag="ft")
                nc.sync.dma_start(
                    out=ft[:],
                    in_=bass.AP(xt, pg * PG * C * BS,
                                [[BS, 128], [C * BS, PG], [128 * BS, 2],
                                 [1, 1024]]))
                f2t = f2tp.tile([128, PG, 2, 1024], bf16, tag="f2t")
                nc.gpsimd.tensor_tensor(
                    out=f2t[:], in0=ft[:], in1=ft[:], op=Alu.mult)
                pg_tiles[pg] = (ft, f2t)
                if pg - 3 in pg_tiles:
                    del pg_tiles[pg - 3]

            iters = [(pg, ibt) for pg in range(NPG) for ibt in range(NBT)]
            st = {}

            def stageA(i):
                pg, ibt = iters[i]
                if ibt == 0:
                    load_pg(pg)
                    load_pg(pg + 1)
                p0, b0 = pg * PG, ibt * 128
                f = fp.tile([128, PG, C], bf16, tag="f")
                nc.sync.dma_start(
                    out=f[:],
                    in_=bass.AP(x, (p0 * BS + b0) * C,
                                [[C, 128], [BS * C, PG], [1, C]]))
                st[i] = {"f": f, "pg": pg, "ibt": ibt}

            def stageB(i):
                s = st[i]
                pg, ibt = s["pg"], s["ibt"]
                f = s["f"]
                ft, f2t = pg_tiles[pg]
                p0 = pg * PG
                # pw1 = f * w_rs  (bf16 TT on DVE)
                pw1 = pw1p.tile([128, PG, C], bf16, tag="pw1")
                nc.vector.tensor_tensor(
                    out=pw1[:], in0=f[:], in1=wB[:, p0:p0 + PG, :],
                    op=Alu.mult)
                # stats: stationary = data tiles, moving = small W columns
                # cols per pt: 0 -> mu, 1 -> E2 (all pre-scaled by 1/C)
                smt = smps.tile([128, 256], f32, tag="smt")
                sm = smt[:, 0:2 * PG]
                for ip in range(PG):
                    pt = p0 + ip
                    for h in range(2):
                        nc.tensor.matmul(
                            smt[:, 2 * ip:2 * ip + 2],
                            ft[:, ip, h, ibt * 128:(ibt + 1) * 128],
                            wMov[:, pt, h, :],
                            start=(h == 0), stop=False,
                            skip_group_check=True)
                        nc.tensor.matmul(
                            smt[:, 2 * ip + 1:2 * ip + 2],
                            f2t[:, ip, h, ibt * 128:(ibt + 1) * 128],
                            wMov2[:, pt, h:h + 1],
                            start=False, stop=False,
                            skip_group_check=True)
                    nc.tensor.matmul(
                        smt[:, 2 * ip:2 * ip + 2], ones1[:], sbRow[:, pt, :],
                        start=False, stop=True, skip_group_check=True)
                s["pw1"], s["smt"] = pw1, smt

            def stageC(i):
                s = st[i]
                smt = s["smt"]
                stat = statp.tile([128, PG, 2], f32, tag="stat")
                nc.vector.tensor_copy(stat[:], smt[:, 0:2 * PG])
                mu = stat[:, :, 0]
                e2 = stat[:, :, 1]
                u = smallp.tile([128, PG], f32, tag="u")
                nc.vector.tensor_tensor(out=u[:], in0=mu, in1=mu, op=Alu.mult)
                v = smallp.tile([128, PG], f32, tag="v")
                nc.vector.scalar_tensor_tensor(
                    out=v[:], in0=e2, scalar=1.0, in1=u[:],
                    op0=Alu.bypass, op1=Alu.subtract)
                w = smallp.tile([128, PG], f32, tag="w")
                nc.scalar.activation(w[:], v[:], Act.Sqrt, bias=eps_t[:])
                r32 = smallp.tile([128, PG], f32, tag="r32")
                nc.vector.reciprocal(r32[:], w[:])
                m2 = smallp.tile([128, PG], f32, tag="m2")
                nc.vector.scalar_tensor_tensor(
                    out=m2[:], in0=mu, scalar=-1.0, in1=r32[:],
                    op0=Alu.mult, op1=Alu.mult)
                # r as a [5, 128] row block for the bias rank-5 accumulate
                rT = s["smt"][0:PG, 64:192]
                nc.tensor.matmul(rT, r32[:], ident[:],
                                 start=True, stop=True,
                                 skip_group_check=True)
                rrow = rrowp.tile([PG, 128], bf16, tag="rrow")
                nc.vector.tensor_copy(rrow[:], rT)
                s["r32"], s["m2"], s["rrow"] = r32, m2, rrow

            def stageD(i):
                s = st[i]
                pg = s["pg"]
                p0 = pg * PG
                f, pw1, r16, m2, rrow = (s["f"], s["pw1"], s["r32"],
                                         s["m2"], s["rrow"])
                # q = pw1 * r + (-r*mu)   per point (DVE 4x tensor_scalar)
                q = qp.tile([128, PG, C], bf16, tag="q")
                for ip in range(PG):
                    nc.vector.tensor_scalar(
                        out=q[:, ip, :], in0=pw1[:, ip, :],
                        scalar1=r16[:, ip:ip + 1], scalar2=m2[:, ip:ip + 1],
                        op0=Alu.mult, op1=Alu.add)
                if has_gamma:
                    for ip in range(PG):
                        nc.vector.tensor_tensor(
                            out=q[:, ip, :], in0=q[:, ip, :], in1=gB[:],
                            op=Alu.mult)
                if has_beta:
                    for ip in range(PG):
                        nc.vector.tensor_tensor(
                            out=q[:, ip, :], in0=q[:, ip, :], in1=bB2[:],
                            op=Alu.add)
                # zT[(pt,h)] = T(q) + T(f) + bias^T (x) r   (f32 PSUM accum)
                zt = ztp.tile([128, PG, 2, 128], f32, tag="zt")
                for ip in range(PG):
                    pt = p0 + ip
                    for h in range(2):
                        o = zt[:, ip, h, :]
                        nc.tensor.matmul(
                            o, q[:, ip, h * 128:(h + 1) * 128], i16[:],
                            start=True, stop=False, skip_group_check=True)
                        nc.tensor.matmul(
                            o, f[:, ip, h * 128:(h + 1) * 128], i16[:],
                            start=False, stop=False, skip_group_check=True)
                        nc.tensor.matmul(
                            o, bT5[:, pt, h, :], rrow[:],
                            start=False, stop=True, skip_group_check=True)
                s["zt"] = zt

            def stageE(i):
                s = st[i]
                pg, ibt = s["pg"], s["ibt"]
                p0, b0 = pg * PG, ibt * 128
                zt = s["zt"]
                # ACT drain with fused Relu -> hT bf16 (two half ops)
                ht = htp.tile([128, PG, 2, 128], bf16, tag="ht")
                nc.scalar.activation(ht[:, 0:3, :, :], zt[:, 0:3, :, :],
                                     Act.Relu)
                nc.scalar.activation(ht[:, 3:PG, :, :], zt[:, 3:PG, :, :],
                                     Act.Relu)
                # proj: stationary hT slice, moving lwT half
                pr = s["smt"][:, 192:192 + PG * D]
                for ip in range(PG):
                    for h in range(2):
                        nc.tensor.matmul(
                            pr[:, ip * D:(ip + 1) * D],
                            ht[:, ip, h, :], lwT[:, h, :],
                            start=(h == 0), stop=(h == 1),
                            skip_group_check=True)
                ost = op_.tile([128, PG, D], f32, tag="ost")
                if has_linb:
                    nc.vector.tensor_tensor(
                        out=ost[:], in0=pr,
                        in1=lbB[:, 0:PG, :], op=Alu.add)
                else:
                    nc.scalar.copy(ost[:], pr)
                nc.scalar.dma_start(
                    out=bass.AP(y, b0 * P * D + p0 * D,
                                [[P * D, 128], [D, PG], [1, D]]),
                    in_=ost[:])
                del st[i]

            n = len(iters)
            for k in range(n + 4):
                if k < n:
                    stageA(k)
                if 0 <= k - 1 < n:
                    stageB(k - 1)
                if 0 <= k - 2 < n:
                    stageC(k - 2)
                if 0 <= k - 3 < n:
                    stageD(k - 3)
                if 0 <= k - 4 < n:
                    stageE(k - 4)

    nc.compile()
    return nc


def _get_nc(has_gamma, has_beta, has_linb):
    key = (has_gamma, has_beta, has_linb)
    if key not in _cache:
        _cache[key] = _build(*key)
    return _cache[key]


def _prep(features, weights, bias, ln_gamma, ln_beta, lin_w, lin_b):
    """Host-side layout prep + tiny parameter folds (no bulk math on f)."""
    import ml_dtypes
    bf = ml_dtypes.bfloat16

    f32 = np.float32
    features = np.asarray(features, f32)
    weights = np.asarray(weights, f32)
    bias = np.asarray(bias, f32)
    lin_w = np.asarray(lin_w, f32)

    wrs = weights.sum(-1)                       # [P, C]
    sb = bias.sum(-1) / C                       # [P]
    sb2 = np.square(bias).sum(-1) / C           # [P]

    # x: [P, BS-per-core, C], xt: [P, C, BS-per-core]
    xp = np.ascontiguousarray(features.transpose(1, 0, 2)).astype(bf)
    xtp = np.ascontiguousarray(features.transpose(1, 2, 0)).astype(bf)

    wrs_h = wrs.reshape(P, 2, 128)              # [pt, h, c]
    bias_h = bias.reshape(P, 2, 128)

    bt5 = np.zeros((5, P, 2, 128), f32)
    for pt in range(P):
        bt5[pt % PG, pt] = bias_h[pt]

    wmv = np.zeros((128, P, 2, 2), f32)
    wmv[:, :, :, 0] = wrs_h.transpose(2, 0, 1) / C
    wmv[:, :, :, 1] = 2.0 * (wrs_h * bias_h).transpose(2, 0, 1) / C
    wmv2 = np.square(wrs_h).transpose(2, 0, 1) / C   # [c, pt, h]

    sbr = np.stack([sb, sb2], axis=-1)          # [P, 2]
    lwt = lin_w.reshape(D, 2, 128).transpose(2, 1, 0)  # [c, h, d]

    common = {
        "wb": wrs.reshape(-1).astype(bf),
        "bt5": bt5.astype(bf),
        "wmv": wmv.astype(bf),
        "wmv2": np.ascontiguousarray(wmv2).astype(bf),
        "sbr": sbr.astype(bf),
        "lwt": np.ascontiguousarray(lwt).astype(bf),
        "g": np.asarray(ln_gamma, f32),
        "bt": np.asarray(ln_beta, f32),
        "lb": np.asarray(lin_b, f32),
    }
    in_maps = []
    for i in range(NCORES):
        m = dict(common)
        m["x"] = np.ascontiguousarray(xp[:, i * BS:(i + 1) * BS, :])
        m["xt"] = np.ascontiguousarray(xtp[:, :, i * BS:(i + 1) * BS])
        in_maps.append(m)
    return in_maps


def kernel(features, weights, bias, ln_gamma, ln_beta, lin_w, lin_b):
    from concourse.bass_utils import run_bass_kernel_spmd

    ln_gamma = np.asarray(ln_gamma, np.float32)
    ln_beta = np.asarray(ln_beta, np.float32)
    lin_b = np.asarray(lin_b, np.float32)
    has_gamma = not np.all(ln_gamma == 1.0)
    has_beta = not np.all(ln_beta == 0.0)
    has_linb = not np.all(lin_b == 0.0)
    nc = _get_nc(has_gamma, has_beta, has_linb)

    in_maps = _prep(features, weights, bias, ln_gamma, ln_beta, lin_w, lin_b)
    res = run_bass_kernel_spmd(nc, in_maps, core_ids=list(range(NCORES)))
    out = np.empty((B, P, D), dtype=np.float32)
    for i in range(NCORES):
        out[i * BS:(i + 1) * BS] = res.results[i]["y"]
    return out



# revision 50
# speedup vs baseline: 1.4391x; 1.4391x over previous
"""Trainium2 Bass kernel for nn_DiseaseHead.

Computation (per the reference):
    w_rs = weights.sum(-1)                       # [P, C]
    pw   = features * w_rs + bias                # [B, P, C]
    pwn  = LayerNorm_c(pw) * gamma + beta
    h    = relu(features + pwn)
    out  = einsum("bpc,dc->bpd", h, lin_w) + lin_b

Sharding: data-parallel over batch B across 8 NeuronCores; parameters
replicated.  All bulk compute in bf16 (tolerance 2e-2).

Per-core structure, per group (128 batch rows x PG=5 points x C=256):
 - f loaded [128b, 5p, 256c] bf16; fT loaded host-pretransposed
   [c, b] bf16 strips per point-group.
 - LN stats via "flip" matmuls: fT / f2T (=fT^2, squared on Pool) as the
   STATIONARY operand, tiny per-point weight columns moving =>
   mu = sum(w_rs*f + bias)/C and E2 = E[pw^2] land directly in [128b, pt]
   partition form (no transpose-back).  sb/sb2 constants enter via a
   ones-stationary rank-1 accumulate.
 - smalls: var = E2 - mu^2, r = 1/sqrt(var+eps) (DVE+ACT), m2 = -r*mu.
 - q = pw1*r + m2 per point on DVE tensor_scalar (4x bf16 mode), where
   pw1 = f*w_rs (DVE TT).  The bias term enters transposed-space as a
   masked rank-5 matmul bias^T (x) r.
 - zT = T(q) + T(f) + bias^T(x)r accumulated in f32 PSUM via regular
   bf16 matmuls (identity moving); ACT drains with fused Relu -> hT bf16.
 - proj: hT as stationary, lin_w^T moving => out[128b, pt*5d] in PSUM;
   ACT copies out; DMA stores [BS, P, D] contiguous 100B rows.
"""

import sys

if "/opt/trn_rl_repo" not in sys.path:
    sys.path.insert(0, "/opt/trn_rl_repo")

import numpy as np

B, P, C, D = 8192, 25, 256, 5
NCORES = 8
BS = B // NCORES          # 1024 batch rows per core
PG = 5                    # points per group
NPG = P // PG             # 5 point-groups
NBT = BS // 128           # 8 batch tiles of 128 rows
LN_EPS = 1e-5

_cache = {}

# schedule knobs (sweepable): iteration slots for prefetch triggers
TRIG_A = 1      # ibt at which chunk_a(pg+2) is issued
TRIG_B = 5      # ibt at which chunk_b(pg+2) is issued
TRIG_B1 = 1     # ibt (pg=0) at which chunk_b(1) is issued


def _build(has_gamma: bool, has_beta: bool, has_linb: bool):
    import concourse.bass as bass
    import concourse.tile as tile
    from concourse import bacc, mybir
    from concourse.masks import make_identity

    f32 = mybir.dt.float32
    bf16 = mybir.dt.bfloat16
    Alu = mybir.AluOpType
    Act = mybir.ActivationFunctionType

    nc = bacc.Bacc("TRN2", target_bir_lowering=False, debug=False,
                   num_devices=NCORES)

    x = nc.dram_tensor("x", [P, BS, C], bf16, kind="ExternalInput")
    xt = nc.dram_tensor("xt", [P, C, BS], bf16, kind="ExternalInput")
    wb = nc.dram_tensor("wb", [P * C], bf16, kind="ExternalInput")
    bt5 = nc.dram_tensor("bt5", [5, P, 2, 128], bf16, kind="ExternalInput")
    wmv = nc.dram_tensor("wmv", [128, P, 2, 2], bf16, kind="ExternalInput")
    wmv2 = nc.dram_tensor("wmv2", [128, P, 2], bf16, kind="ExternalInput")
    sbb = nc.dram_tensor("sbb", [P * 2], f32, kind="ExternalInput")
    lwt = nc.dram_tensor("lwt", [128, 2, D], bf16, kind="ExternalInput")
    g = nc.dram_tensor("g", [C], f32, kind="ExternalInput")
    bt_ = nc.dram_tensor("bt", [C], f32, kind="ExternalInput")
    lb = nc.dram_tensor("lb", [D], f32, kind="ExternalInput")
    y = nc.dram_tensor("y", [BS, P, D], f32, kind="ExternalOutput")

    with tile.TileContext(nc) as tc:
        with (
            tc.tile_pool(name="consts", bufs=1) as consts,
            tc.tile_pool(name="ftp", bufs=3) as ftp,
            tc.tile_pool(name="f2tp", bufs=3) as f2tp,
            tc.tile_pool(name="fp", bufs=6) as fp,
            tc.tile_pool(name="pw1p", bufs=4) as pw1p,
            tc.tile_pool(name="qp", bufs=4) as qp,
            tc.tile_pool(name="htp", bufs=3) as htp,
            tc.tile_pool(name="statp", bufs=4) as statp,
            tc.tile_pool(name="smallp", bufs=16) as smallp,
            tc.tile_pool(name="rrowp", bufs=4) as rrowp,
            tc.tile_pool(name="op", bufs=4) as op_,
            tc.tile_pool(name="zt", bufs=2, space="PSUM") as ztp,
            tc.tile_pool(name="smps", bufs=2, space="PSUM") as smps,
        ):
            ident = consts.tile([128, 128], f32)
            make_identity(nc, ident[:])
            i16 = consts.tile([128, 128], bf16)
            nc.vector.tensor_copy(i16[:], ident[:])

            eps_t = consts.tile([128, 1], f32)
            nc.vector.memset(eps_t[:], LN_EPS)

            bT5 = consts.tile([5, P, 2, 128], bf16)
            nc.sync.dma_start(out=bT5[:], in_=bt5[:])
            wMov = consts.tile([128, P, 2, 2], bf16)
            nc.sync.dma_start(out=wMov[:], in_=wmv[:])
            wMov2 = consts.tile([128, P, 2], bf16)
            nc.sync.dma_start(out=wMov2[:], in_=wmv2[:])
            sbB = consts.tile([128, NPG, PG, 2], f32)
            nc.sync.dma_start(
                out=sbB[:], in_=bass.AP(sbb, 0, [[0, 128], [2 * PG, NPG],
                                                 [2, PG], [1, 2]]))
            lwT = consts.tile([128, 2, D], bf16)
            nc.sync.dma_start(out=lwT[:], in_=lwt[:])

            if has_gamma:
                gB = consts.tile([128, C], bf16)
                nc.gpsimd.dma_start(
                    out=gB[:], in_=bass.AP(g, 0, [[0, 128], [1, C]]))
            if has_beta:
                bB2 = consts.tile([128, C], bf16)
                nc.gpsimd.dma_start(
                    out=bB2[:], in_=bass.AP(bt_, 0, [[0, 128], [1, C]]))
            if has_linb:
                lbB = consts.tile([128, PG, D], f32)
                nc.gpsimd.dma_start(
                    out=lbB[:], in_=bass.AP(lb, 0, [[0, 128], [0, PG], [1, D]]))

            # per-b-chunk output accumulator: filled across the 5 point
            # groups, stored once per b-chunk (8 big stores vs 40 small)
            yacc = consts.tile([128, NBT, NPG, PG, D], f32)

            pg_tiles = {}

            def _xt_dma(ft, pg, a, b):
                nc.sync.dma_start(
                    out=ft[:, a:b],
                    in_=bass.AP(xt, (pg * PG + a) * C * BS,
                                [[BS, 128], [C * BS, b - a],
                                 [128 * BS, 2], [1, 1024]]))

            def chunk_a(pg, split=False):
                """Allocate pg's tiles and load points 3:5 (whose squaring
                slots come first).  Chunked so no 7.3us transfer head-blocks
                the per-iter f loads on the (serialized) DMA engine pool."""
                if pg in pg_tiles or pg >= NPG:
                    return
                ft = ftp.tile([128, PG, 2, 1024], bf16, tag="ft")
                for a, b in ((3, 4), (4, PG)) if split else ((3, PG),):
                    _xt_dma(ft, pg, a, b)
                f2t = f2tp.tile([128, PG, 2, 1024], bf16, tag="f2t")
                pg_tiles[pg] = (ft, f2t)
                if pg - 3 in pg_tiles:
                    del pg_tiles[pg - 3]

            def chunk_b(pg, split=False):
                if pg >= NPG:
                    return
                ft = pg_tiles[pg][0]
                for a, b in ((0, 1), (1, 2), (2, 3)) if split else ((0, 3),):
                    _xt_dma(ft, pg, a, b)

            def square_point(pg, j, eng=None):
                """Emit the f^2 op for point j of group pg.  Issued late
                (spread one per iteration) so a not-yet-loaded ft tile never
                head-blocks an engine's in-order queue; split DVE / ACT /
                Pool so no engine takes a 20us serial block."""
                if pg >= NPG:
                    return
                ft, f2t = pg_tiles[pg]
                sl = slice(j, j + 1)
                if eng is None:
                    eng = "v" if j < 2 else ("a" if j == 2 else "p")
                if eng == "v":
                    nc.vector.tensor_tensor(
                        out=f2t[:, sl], in0=ft[:, sl], in1=ft[:, sl],
                        op=Alu.mult)
                elif eng == "a":
                    nc.scalar.activation(f2t[:, sl], ft[:, sl], Act.Square)
                else:
                    nc.gpsimd.tensor_tensor(
                        out=f2t[:, sl], in0=ft[:, sl], in1=ft[:, sl],
                        op=Alu.mult)

            iters = [(pg, ibt) for pg in range(NPG) for ibt in range(NBT)]
            st = {}

            def stageA(i):
                pg, ibt = iters[i]
                p0, b0 = pg * PG, ibt * 128
                # the latency-critical per-iter f load goes FIRST; bulk
                # prefetches for later groups queue behind it
                f = fp.tile([128, PG, C], bf16, tag="f")
                nc.sync.dma_start(
                    out=f[:],
                    in_=bass.AP(x, (p0 * BS + b0) * C,
                                [[C, 128], [BS * C, PG], [1, C]]))
                st[i] = {"f": f, "pg": pg, "ibt": ibt}
                # just-in-time prefetch: each chunk lands ~2 iters before
                # its first consumer, never sooner (frees early bandwidth
                # for the latency-critical f loads)
                if pg == 0 and ibt == TRIG_B1:
                    chunk_b(1)
                if ibt == TRIG_A:
                    chunk_a(pg + 2)
                    if pg + 1 < NPG:
                        load_wb(pg + 1)
                if ibt == TRIG_B:
                    chunk_b(pg + 2)

            def stageB(i):
                s = st[i]
                pg, ibt = s["pg"], s["ibt"]
                ft, f2t = pg_tiles[pg]
                p0 = pg * PG
                # stats: stationary = data tiles, moving = small W columns
                # cols per pt: 0 -> mu, 1 -> E2 (all pre-scaled by 1/C)
                smt = smps.tile([128, 256], f32, tag="smt")
                for ip in range(PG):
                    pt = p0 + ip
                    bs = slice(ibt * 128, (ibt + 1) * 128)
                    nc.tensor.matmul(
                        smt[:, 2 * ip:2 * ip + 2], ft[:, ip, 0, bs],
                        wMov[:, pt, 0, :], start=True, stop=False,
                        skip_group_check=True)
                    nc.tensor.matmul(
                        smt[:, 2 * ip + 1:2 * ip + 2], f2t[:, ip, 0, bs],
                        wMov2[:, pt, 0:1], start=False, stop=False,
                        skip_group_check=True)
                    nc.tensor.matmul(
                        smt[:, 2 * ip + 1:2 * ip + 2], f2t[:, ip, 1, bs],
                        wMov2[:, pt, 1:2], start=False, stop=False,
                        skip_group_check=True)
                    nc.tensor.matmul(
                        smt[:, 2 * ip:2 * ip + 2], ft[:, ip, 1, bs],
                        wMov[:, pt, 1, :], start=False, stop=True,
                        skip_group_check=True)
                s["smt"] = smt

            def stageC(i):
                s = st[i]
                smt = s["smt"]
                stat = statp.tile([128, PG, 2], f32, tag="stat")
                nc.vector.tensor_tensor(
                    out=stat[:], in0=smt[:, 0:2 * PG], in1=sbB[:, s["pg"]],
                    op=Alu.add)
                mu = stat[:, :, 0]
                e2 = stat[:, :, 1]
                u = smallp.tile([128, PG], f32, tag="u")
                nc.vector.tensor_tensor(out=u[:], in0=mu, in1=mu, op=Alu.mult)
                v = smallp.tile([128, PG], f32, tag="v")
                nc.vector.scalar_tensor_tensor(
                    out=v[:], in0=e2, scalar=1.0, in1=u[:],
                    op0=Alu.bypass, op1=Alu.subtract)
                w = smallp.tile([128, PG], f32, tag="w")
                nc.scalar.activation(w[:], v[:], Act.Sqrt, bias=eps_t[:])
                r32 = smallp.tile([128, PG], f32, tag="r32")
                nc.vector.reciprocal(r32[:], w[:])
                m2 = smallp.tile([128, PG], f32, tag="m2")
                nc.vector.scalar_tensor_tensor(
                    out=m2[:], in0=mu, scalar=-1.0, in1=r32[:],
                    op0=Alu.mult, op1=Alu.mult)
                # r as a [5, 128] row block for the bias rank-5 accumulate
                rT = s["smt"][0:PG, 64:192]
                nc.tensor.matmul(rT, r32[:], ident[:],
                                 start=True, stop=True,
                                 skip_group_check=True)
                rrow = rrowp.tile([PG, 128], bf16, tag="rrow")
                nc.vector.tensor_copy(rrow[:], rT)
                s["r32"], s["m2"], s["rrow"] = r32, m2, rrow

            def stageD(i):
                s = st[i]
                pg = s["pg"]
                p0 = pg * PG
                f, r16, m2, rrow = (s["f"], s["r32"], s["m2"], s["rrow"])
                # pw1 = f * w_rs (bf16 TT on DVE) -- issued here, not in
                # stageB, so it sits behind the latency-critical smalls in
                # the in-order DVE queue rather than ahead of them
                pw1 = pw1p.tile([128, PG, C], bf16, tag="pw1")
                nc.vector.tensor_tensor(
                    out=pw1[:], in0=f[:], in1=wB[:, p0:p0 + PG, :],
                    op=Alu.mult)
                # q = pw1 * r + (-r*mu)   per point (DVE 4x tensor_scalar)
                q = qp.tile([128, PG, C], bf16, tag="q")
                for ip in range(PG):
                    nc.vector.tensor_scalar(
                        out=q[:, ip, :], in0=pw1[:, ip, :],
                        scalar1=r16[:, ip:ip + 1], scalar2=m2[:, ip:ip + 1],
                        op0=Alu.mult, op1=Alu.add)
                if has_gamma:
                    for ip in range(PG):
                        nc.vector.tensor_tensor(
                            out=q[:, ip, :], in0=q[:, ip, :], in1=gB[:],
                            op=Alu.mult)
                if has_beta:
                    for ip in range(PG):
                        nc.vector.tensor_tensor(
                            out=q[:, ip, :], in0=q[:, ip, :], in1=bB2[:],
                            op=Alu.add)
                # zT[(pt,h)] = T(q) + T(f) + bias^T (x) r   (f32 PSUM accum)
                # extra 6th "point" slot: scratch for the proj output (the
                # pool rounds to 3 banks anyway; same D->E lifetime)
                zt = ztp.tile([128, PG + 1, 2, 128], f32, tag="zt")
                for ip in range(PG):
                    pt = p0 + ip
                    for h in range(2):
                        o = zt[:, ip, h, :]
                        nc.tensor.matmul(
                            o, q[:, ip, h * 128:(h + 1) * 128], i16[:],
                            start=True, stop=False, skip_group_check=True)
                        nc.tensor.matmul(
                            o, f[:, ip, h * 128:(h + 1) * 128], i16[:],
                            start=False, stop=False, skip_group_check=True)
                        nc.tensor.matmul(
                            o, bT5[:, pt, h, :], rrow[:],
                            start=False, stop=True, skip_group_check=True)
                s["zt"] = zt

            def stageE(i):
                s = st[i]
                pg, ibt = s["pg"], s["ibt"]
                p0, b0 = pg * PG, ibt * 128
                zt = s["zt"]
                # ACT drain with fused Relu -> hT bf16
                ht = htp.tile([128, PG, 2, 128], bf16, tag="ht")
                nc.scalar.activation(ht[:], zt[:, 0:PG, :, :], Act.Relu)
                # proj: stationary hT slice, moving lwT half
                pr = zt[:, PG, 0, 0:PG * D]
                for ip in range(PG):
                    for h in range(2):
                        nc.tensor.matmul(
                            pr[:, ip * D:(ip + 1) * D],
                            ht[:, ip, h, :], lwT[:, h, :],
                            start=(h == 0), stop=(h == 1),
                            skip_group_check=True)
                ost = yacc[:, ibt, pg]
                if has_linb:
                    nc.vector.tensor_tensor(
                        out=ost, in0=pr,
                        in1=lbB[:, 0:PG, :], op=Alu.add)
                else:
                    nc.scalar.copy(ost, pr)
                if pg == NPG - 1:
                    nc.gpsimd.dma_start(
                        out=bass.AP(y, b0 * P * D,
                                    [[P * D, 128], [1, P * D]]),
                        in_=yacc[:, ibt])
                del st[i]

            # prologue: pg0 loads per-point + squares on the fast engines
            # (DVE/ACT only -- Pool's 4.3us ops would gate the first stats);
            # wB comes per-group, first slice right after pg0's data
            chunk_a(0, split=True)
            chunk_b(0, split=True)
            for j, eng in ((3, "v"), (4, "a"), (0, "v"), (1, "a"), (2, "v")):
                square_point(0, j, eng)
            wB = consts.tile([128, P, C], bf16)

            def load_wb(pg):
                nc.sync.dma_start(
                    out=wB[:, pg * PG:(pg + 1) * PG, :],
                    in_=bass.AP(wb, pg * PG * C, [[0, 128], [1, PG * C]]))

            load_wb(0)
            chunk_a(1)

            # square-op spread: at iteration (pg, ibt) emit one f^2 op for
            # pg+1 (its xt data was prefetched a full group earlier);
            # points 3,4 (first DMA chunk) earliest, on Pool
            spread = {1: 3, 2: 4, 4: 0, 5: 1, 6: 2}

            n = len(iters)
            for k in range(n + 4):
                if k < n:
                    stageA(k)
                if 0 <= k - 1 < n:
                    stageB(k - 1)
                if 0 <= k - 2 < n:
                    stageC(k - 2)
                if 0 <= k - 3 < n:
                    stageD(k - 3)
                if 0 <= k - 4 < n:
                    stageE(k - 4)
                if k < n:
                    pg, ibt = iters[k]
                    if ibt in spread:
                        square_point(pg + 1, spread[ibt])

    nc.compile()
    return nc


def _get_nc(has_gamma, has_beta, has_linb):
    key = (has_gamma, has_beta, has_linb)
    if key not in _cache:
        _cache[key] = _build(*key)
    return _cache[key]


def _prep(features, weights, bias, ln_gamma, ln_beta, lin_w, lin_b):
    """Host-side layout prep + tiny parameter folds (no bulk math on f)."""
    import ml_dtypes
    bf = ml_dtypes.bfloat16

    f32 = np.float32
    features = np.asarray(features, f32)
    weights = np.asarray(weights, f32)
    bias = np.asarray(bias, f32)
    lin_w = np.asarray(lin_w, f32)

    wrs = weights.sum(-1)                       # [P, C]
    sb = bias.sum(-1) / C                       # [P]
    sb2 = np.square(bias).sum(-1) / C           # [P]

    # x: [P, BS-per-core, C], xt: [P, C, BS-per-core]
    xp = np.ascontiguousarray(features.transpose(1, 0, 2)).astype(bf)
    xtp = np.ascontiguousarray(features.transpose(1, 2, 0)).astype(bf)

    wrs_h = wrs.reshape(P, 2, 128)              # [pt, h, c]
    bias_h = bias.reshape(P, 2, 128)

    bt5 = np.zeros((5, P, 2, 128), f32)
    for pt in range(P):
        bt5[pt % PG, pt] = bias_h[pt]

    wmv = np.zeros((128, P, 2, 2), f32)
    wmv[:, :, :, 0] = wrs_h.transpose(2, 0, 1) / C
    wmv[:, :, :, 1] = 2.0 * (wrs_h * bias_h).transpose(2, 0, 1) / C
    wmv2 = np.square(wrs_h).transpose(2, 0, 1) / C   # [c, pt, h]

    sbb = np.stack([sb, sb2], axis=-1).reshape(-1)   # [P*2]
    lwt = lin_w.reshape(D, 2, 128).transpose(2, 1, 0)  # [c, h, d]

    common = {
        "wb": wrs.reshape(-1).astype(bf),
        "bt5": bt5.astype(bf),
        "wmv": wmv.astype(bf),
        "wmv2": np.ascontiguousarray(wmv2).astype(bf),
        "sbb": sbb.astype(f32),
        "lwt": np.ascontiguousarray(lwt).astype(bf),
        "g": np.asarray(ln_gamma, f32),
        "bt": np.asarray(ln_beta, f32),
        "lb": np.asarray(lin_b, f32),
    }
    in_maps = []
    for i in range(NCORES):
        m = dict(common)
        m["x"] = np.ascontiguousarray(xp[:, i * BS:(i + 1) * BS, :])
        m["xt"] = np.ascontiguousarray(xtp[:, :, i * BS:(i + 1) * BS])
        in_maps.append(m)
    return in_maps


def kernel(features, weights, bias, ln_gamma, ln_beta, lin_w, lin_b):
    from concourse.bass_utils import run_bass_kernel_spmd

    ln_gamma = np.asarray(ln_gamma, np.float32)
    ln_beta = np.asarray(ln_beta, np.float32)
    lin_b = np.asarray(lin_b, np.float32)
    has_gamma = not np.all(ln_gamma == 1.0)
    has_beta = not np.all(ln_beta == 0.0)
    has_linb = not np.all(lin_b == 0.0)
    nc = _get_nc(has_gamma, has_beta, has_linb)

    in_maps = _prep(features, weights, bias, ln_gamma, ln_beta, lin_w, lin_b)
    res = run_bass_kernel_spmd(nc, in_maps, core_ids=list(range(NCORES)))
    out = np.empty((B, P, D), dtype=np.float32)
    for i in range(NCORES):
        out[i * BS:(i + 1) * BS] = res.results[i]["y"]
    return out



# revision 88
# speedup vs baseline: 1.7094x; 1.1879x over previous
"""Trainium2 Bass kernel for nn_DiseaseHead.

Computation (per the reference):
    w_rs = weights.sum(-1)                       # [P, C]
    pw   = features * w_rs + bias                # [B, P, C]
    pwn  = LayerNorm_c(pw) * gamma + beta
    h    = relu(features + pwn)
    out  = einsum("bpc,dc->bpd", h, lin_w) + lin_b

Sharding: data-parallel over batch B across 8 NeuronCores; parameters
replicated.  Main datapath bf16; the LN-stats side band runs on an fp8
copy of the features (tolerance 2e-2).

Structure (per core: BS=1024 rows, 25 points in 5 groups of PG=5):
 - x loaded [128b, 5p, 256c] bf16 per iteration (pg, b-chunk); xt loaded
   fp8 [c, b] strips per point-group, prefetched ~1.5 groups ahead.
 - LN stats run a FULL GROUP AHEAD of the main pipeline: fT / fT^2 as
   PE stationary, unscaled fp8 weight columns moving; all 8 b-chunks of
   a group accumulate into one PSUM tile.  One batched read-out applies
   1/C + the bias constants, then var/rsqrt/m2 on [128, 40] tiles and a
   single [40, 128] transpose of r for the rank-matmul side.
 - per iteration: pw1 = f*w_rs (DVE tt), q = pw1*r + (-r*mu) (DVE 4x
   tensor_scalar), zT = T(q) + T(f) + bias^T (x) r in PSUM (identity /
   rrow moving), ACT drains with fused Relu, proj against lin_w^T, out
   rows accumulate in SBUF and store once per b-chunk.
"""

import sys

if "/opt/trn_rl_repo" not in sys.path:
    sys.path.insert(0, "/opt/trn_rl_repo")

import numpy as np

B, P, C, D = 8192, 25, 256, 5
NCORES = 8
BS = B // NCORES          # 1024 batch rows per core
PG = 5                    # points per group
NPG = P // PG             # 5 point-groups
NBT = BS // 128           # 8 batch tiles of 128 rows
LN_EPS = 1e-5

_cache = {}

# schedule knobs (sweepable): iteration slots for prefetch triggers
TRIG_A = 2      # ibt at which chunk_a(pg+2) is issued
TRIG_B = 6      # ibt at which chunk_b(pg+2) is issued
TRIG_B1 = 1     # ibt (pg=0) at which chunk_b(1) is issued
# f^2 engine per point and issue slots (for pg+1, during pg); stats
# bursts are timed one slot after their square completes so the waiting
# matmuls never head-block the in-order PE queue
SQ_ENG = {0: "v", 1: "p", 2: "a", 3: "p", 4: "p"}
SQ_SLOT = {0: 3, 1: 4, 2: 2, 3: 0, 4: 1}
ST_SLOT = {3: 3, 4: 2, 5: 4, 6: 0, 7: 1}


def _build(has_gamma: bool, has_beta: bool, has_linb: bool):
    import concourse.bass as bass
    import concourse.tile as tile
    from concourse import bacc, mybir
    from concourse.masks import make_identity

    f32 = mybir.dt.float32
    bf16 = mybir.dt.bfloat16
    f8 = mybir.dt.float8e4
    Alu = mybir.AluOpType
    Act = mybir.ActivationFunctionType

    nc = bacc.Bacc("TRN2", target_bir_lowering=False, debug=False,
                   num_devices=NCORES)

    x = nc.dram_tensor("x", [P, BS, C], bf16, kind="ExternalInput")
    xt = nc.dram_tensor("xt", [P, C, BS], f8, kind="ExternalInput")
    wb = nc.dram_tensor("wb", [P * C], bf16, kind="ExternalInput")
    bt5 = nc.dram_tensor("bt5", [5, P, 2, 128], bf16, kind="ExternalInput")
    wmv = nc.dram_tensor("wmv", [128, P, 2, 2], f8, kind="ExternalInput")
    wmv2 = nc.dram_tensor("wmv2", [128, P, 2], f8, kind="ExternalInput")
    sbb = nc.dram_tensor("sbb", [P * 2], f32, kind="ExternalInput")
    lwt = nc.dram_tensor("lwt", [128, 2, D], bf16, kind="ExternalInput")
    g = nc.dram_tensor("g", [C], f32, kind="ExternalInput")
    bt_ = nc.dram_tensor("bt", [C], f32, kind="ExternalInput")
    lb = nc.dram_tensor("lb", [D], f32, kind="ExternalInput")
    y = nc.dram_tensor("y", [BS, P, D], f32, kind="ExternalOutput")

    with tile.TileContext(nc) as tc:
        with (
            tc.tile_pool(name="consts", bufs=1) as consts,
            tc.tile_pool(name="ftp", bufs=3) as ftp,
            tc.tile_pool(name="f2tp", bufs=3) as f2tp,
            tc.tile_pool(name="fp", bufs=8) as fp,
            tc.tile_pool(name="pw1p", bufs=4) as pw1p,
            tc.tile_pool(name="qp", bufs=4) as qp,
            tc.tile_pool(name="htp", bufs=3) as htp,
            tc.tile_pool(name="statp", bufs=2) as statp,
            tc.tile_pool(name="smallp", bufs=8) as smallp,
            tc.tile_pool(name="rrowp", bufs=4) as rrowp,
            tc.tile_pool(name="zt", bufs=2, space="PSUM") as ztp,
            tc.tile_pool(name="smps", bufs=2, space="PSUM") as smps,
        ):
            ident = consts.tile([128, 128], f32)
            make_identity(nc, ident[:])
            i16 = consts.tile([128, 128], bf16)
            nc.vector.tensor_copy(i16[:], ident[:])

            eps_t = consts.tile([128, 1], f32)
            nc.vector.memset(eps_t[:], LN_EPS)

            bT5 = consts.tile([5, P, 2, 128], bf16)
            wMov = consts.tile([128, P, 2, 2], f8)
            nc.sync.dma_start(out=wMov[:], in_=wmv[:])
            wMov2 = consts.tile([128, P, 2], f8)
            nc.sync.dma_start(out=wMov2[:], in_=wmv2[:])
            # sb/sb2 constants broadcast over partitions AND b-chunks
            # (loaded after pg0's data -- first needed at cbatch(0))
            sbB = consts.tile([128, NBT, NPG, PG, 2], f32)
            lwT = consts.tile([128, 2, D], bf16)

            if has_gamma:
                gB = consts.tile([128, C], bf16)
                nc.gpsimd.dma_start(
                    out=gB[:], in_=bass.AP(g, 0, [[0, 128], [1, C]]))
            if has_beta:
                bB2 = consts.tile([128, C], bf16)
                nc.gpsimd.dma_start(
                    out=bB2[:], in_=bass.AP(bt_, 0, [[0, 128], [1, C]]))
            if has_linb:
                lbB = consts.tile([128, PG, D], f32)
                nc.gpsimd.dma_start(
                    out=lbB[:], in_=bass.AP(lb, 0, [[0, 128], [0, PG], [1, D]]))

            # per-b-chunk output accumulator: filled across the 5 point
            # groups, stored once per b-chunk (8 big stores vs 40 small)
            yacc = consts.tile([128, NBT, NPG, PG, D], f32)

            pg_tiles = {}
            pg_smt = {}
            pg_stats = {}

            def _xt_dma(ft, pg, a, b):
                nc.sync.dma_start(
                    out=ft[:, a:b],
                    in_=bass.AP(xt, (pg * PG + a) * C * BS,
                                [[BS, 128], [C * BS, b - a],
                                 [128 * BS, 2], [1, 1024]]))

            def chunk_a(pg, split=False):
                """Allocate pg's tiles and load points 3:5 (whose squaring
                slots come first).  Chunked so no single transfer
                head-blocks the per-iter f loads on the DMA engine pool."""
                if pg in pg_tiles or pg >= NPG:
                    return
                ft = ftp.tile([128, PG, 2, 1024], f8, tag="ft")
                for a, b in ((3, 4), (4, PG)) if split else ((3, PG),):
                    _xt_dma(ft, pg, a, b)
                f2t = f2tp.tile([128, PG, 2, 1024], f8, tag="f2t")
                pg_tiles[pg] = (ft, f2t)
                if pg - 3 in pg_tiles:
                    del pg_tiles[pg - 3]

            def get_smt(pg):
                """Per-group stats PSUM, allocated lazily at the first
                stats burst: cols 0:80 = [NBT, 2*PG] accum; cols 80:208 =
                per-iteration rT scratch (keeps the r-transpose chain off
                the zt-recycle loop)."""
                if pg not in pg_smt:
                    smt = smps.tile([128, 208], f32, tag="smt")
                    pg_smt[pg] = smt
                    if pg - 2 in pg_smt:
                        del pg_smt[pg - 2]
                return pg_smt[pg]

            def chunk_b(pg, split=False):
                if pg >= NPG:
                    return
                ft = pg_tiles[pg][0]
                for a, b in ((0, 1), (1, 2), (2, 3)) if split else ((0, 3),):
                    _xt_dma(ft, pg, a, b)

            def square_point(pg, j, eng=None):
                """Emit the f^2 op for point j of group pg, spread one per
                iteration and split across DVE/ACT/Pool."""
                if pg >= NPG:
                    return
                ft, f2t = pg_tiles[pg]
                sl = slice(j, j + 1)
                eng = eng or SQ_ENG[j]
                if eng == "v":
                    nc.vector.tensor_tensor(
                        out=f2t[:, sl], in0=ft[:, sl], in1=ft[:, sl],
                        op=Alu.mult)
                elif eng == "a":
                    nc.scalar.activation(f2t[:, sl], ft[:, sl], Act.Square)
                else:
                    nc.gpsimd.tensor_tensor(
                        out=f2t[:, sl], in0=ft[:, sl], in1=ft[:, sl],
                        op=Alu.mult)

            def stats_point(pg, j):
                """Stats matmuls for point j of pg, ALL 8 b-chunks -- runs
                a full group ahead (only needs the prefetched ft/f2t).
                Cols per point: 0 -> mu-sum, 1 -> E2-sum (unscaled)."""
                if pg >= NPG:
                    return
                ft, f2t = pg_tiles[pg]
                smt = get_smt(pg)
                pt = pg * PG + j
                for ibt in range(NBT):
                    bs = slice(ibt * 128, (ibt + 1) * 128)
                    c0 = ibt * 2 * PG + 2 * j
                    nc.tensor.matmul(
                        smt[:, c0:c0 + 2], ft[:, j, 0, bs],
                        wMov[:, pt, 0, :], start=True, stop=False,
                        skip_group_check=True)
                    nc.tensor.matmul(
                        smt[:, c0 + 1:c0 + 2], f2t[:, j, 0, bs],
                        wMov2[:, pt, 0:1], start=False, stop=False,
                        skip_group_check=True)
                    nc.tensor.matmul(
                        smt[:, c0 + 1:c0 + 2], f2t[:, j, 1, bs],
                        wMov2[:, pt, 1:2], start=False, stop=False,
                        skip_group_check=True)
                    nc.tensor.matmul(
                        smt[:, c0:c0 + 2], ft[:, j, 1, bs],
                        wMov[:, pt, 1, :], start=False, stop=True,
                        skip_group_check=True)

            def cbatch(pg):
                """Batched LN smalls for the whole group [128, NBT, PG]:
                one read-out (applies 1/C + sb consts), var, rsqrt, m2,
                and ONE [40, 128] transpose of r."""
                if pg >= NPG:
                    return
                smt = pg_smt[pg]
                stat = statp.tile([128, NBT, PG, 2], f32, tag="stat")
                nc.vector.scalar_tensor_tensor(
                    out=stat[:], in0=smt[:, 0:NBT * 2 * PG], scalar=1.0 / C,
                    in1=sbB[:, :, pg], op0=Alu.mult, op1=Alu.add)
                mu = stat[:, :, :, 0]
                e2 = stat[:, :, :, 1]
                u = smallp.tile([128, NBT, PG], f32, tag="u")
                nc.vector.tensor_tensor(out=u[:], in0=mu, in1=mu, op=Alu.mult)
                v = smallp.tile([128, NBT, PG], f32, tag="v")
                nc.vector.scalar_tensor_tensor(
                    out=v[:], in0=e2, scalar=1.0, in1=u[:],
                    op0=Alu.bypass, op1=Alu.subtract)
                w = smallp.tile([128, NBT, PG], f32, tag="w")
                nc.scalar.activation(w[:], v[:], Act.Sqrt, bias=eps_t[:])
                r32 = smallp.tile([128, NBT, PG], f32, tag="r32")
                nc.vector.reciprocal(r32[:], w[:])
                m2 = smallp.tile([128, NBT, PG], f32, tag="m2")
                nc.vector.scalar_tensor_tensor(
                    out=m2[:], in0=mu, scalar=-1.0, in1=r32[:],
                    op0=Alu.mult, op1=Alu.mult)
                # bf16 copy of r for the cheap per-iter [5,128] transpose
                rb = smallp.tile([128, NBT, PG], bf16, tag="rb")
                nc.vector.tensor_copy(rb[:], r32[:])
                pg_stats[pg] = (r32, m2, rb)
                if pg - 2 in pg_stats:
                    del pg_stats[pg - 2]

            iters = [(pg, ibt) for pg in range(NPG) for ibt in range(NBT)]
            st = {}
            rrows = {}

            def stageA(i):
                pg, ibt = iters[i]
                p0, b0 = pg * PG, ibt * 128
                # the latency-critical per-iter f load goes FIRST; bulk
                # prefetches for later groups queue behind it
                f = fp.tile([128, PG, C], bf16, tag="f")
                nc.sync.dma_start(
                    out=f[:],
                    in_=bass.AP(x, (p0 * BS + b0) * C,
                                [[C, 128], [BS * C, PG], [1, C]]))
                st[i] = {"f": f, "pg": pg, "ibt": ibt}
                if pg == 0 and ibt == TRIG_B1:
                    chunk_b(1)
                if ibt == TRIG_A:
                    chunk_a(pg + 2)
                    if pg + 1 < NPG:
                        load_wb(pg + 1)
                if ibt == TRIG_B:
                    chunk_b(pg + 2)

            def stageD(i):
                s = st[i]
                pg, ibt = s["pg"], s["ibt"]
                p0 = pg * PG
                f = s["f"]
                r32, m2, rb = pg_stats[pg]
                # pw1 = f * w_rs  (bf16 TT on DVE)
                pw1 = pw1p.tile([128, PG, C], bf16, tag="pw1")
                nc.vector.tensor_tensor(
                    out=pw1[:], in0=f[:], in1=wB[:, p0:p0 + PG, :],
                    op=Alu.mult)
                # q = pw1 * r + (-r*mu)   per point (DVE 4x tensor_scalar)
                q = qp.tile([128, PG, C], bf16, tag="q")
                for ip in range(PG):
                    nc.vector.tensor_scalar(
                        out=q[:, ip, :], in0=pw1[:, ip, :],
                        scalar1=r32[:, ibt, ip:ip + 1],
                        scalar2=m2[:, ibt, ip:ip + 1],
                        op0=Alu.mult, op1=Alu.add)
                if has_gamma:
                    for ip in range(PG):
                        nc.vector.tensor_tensor(
                            out=q[:, ip, :], in0=q[:, ip, :], in1=gB[:],
                            op=Alu.mult)
                if has_beta:
                    for ip in range(PG):
                        nc.vector.tensor_tensor(
                            out=q[:, ip, :], in0=q[:, ip, :], in1=bB2[:],
                            op=Alu.add)
                # zT[(pt,h)] = T(q) + T(f) + bias^T (x) r   (f32 PSUM accum)
                # extra 6th "point" slot: proj output + rT scratch (the
                # pool rounds to 3 banks anyway; same D->E lifetime)
                zt = ztp.tile([128, PG + 1, 2, 128], f32, tag="zt")
                rT = pg_smt[pg][0:PG, 80:208]
                nc.tensor.matmul(rT, rb[:, ibt, :], i16[:],
                                 start=True, stop=True,
                                 skip_group_check=True)
                rrow = rrowp.tile([PG, 128], bf16, tag="rrow")
                nc.vector.tensor_copy(rrow[:], rT)
                for ip in range(PG):
                    pt = p0 + ip
                    for h in range(2):
                        nc.tensor.matmul(
                            zt[:, ip, h, :],
                            q[:, ip, h * 128:(h + 1) * 128], i16[:],
                            start=True, stop=False, skip_group_check=True)
                        nc.tensor.matmul(
                            zt[:, ip, h, :],
                            f[:, ip, h * 128:(h + 1) * 128], i16[:],
                            start=False, stop=False, skip_group_check=True)
                        nc.tensor.matmul(
                            zt[:, ip, h, :], bT5[0:PG, pt, h, :], rrow[:],
                            start=False, stop=True, skip_group_check=True)
                s["zt"] = zt

            def stageE(i):
                s = st[i]
                pg, ibt = s["pg"], s["ibt"]
                b0 = ibt * 128
                zt = s["zt"]
                # ACT drain with fused Relu -> hT bf16
                ht = htp.tile([128, PG, 2, 128], bf16, tag="ht")
                nc.scalar.activation(ht[:], zt[:, 0:PG, :, :], Act.Relu)
                # proj: stationary hT slice, moving lwT half
                pr = zt[:, PG, 0, 0:PG * D]
                for ip in range(PG):
                    for h in range(2):
                        nc.tensor.matmul(
                            pr[:, ip * D:(ip + 1) * D],
                            ht[:, ip, h, :], lwT[:, h, :],
                            start=(h == 0), stop=(h == 1),
                            skip_group_check=True)
                ost = yacc[:, ibt, pg]
                if has_linb:
                    nc.vector.tensor_tensor(
                        out=ost, in0=pr,
                        in1=lbB[:, 0:PG, :], op=Alu.add)
                else:
                    nc.scalar.copy(ost, pr)
                if pg == NPG - 1:
                    nc.gpsimd.dma_start(
                        out=bass.AP(y, b0 * P * D,
                                    [[P * D, 128], [1, P * D]]),
                        in_=yacc[:, ibt])
                del st[i]

            # PE p-state warmup: ~3us of dependency-free matmuls so the
            # first real transposes run at full clock (scratch: rT region)
            wsc = smps.tile([128, 208], f32, tag="smt")
            for _ in range(48):
                nc.tensor.matmul(wsc[:, 80:208], i16[:], i16[:],
                                 start=True, stop=True,
                                 skip_group_check=True)

            # prologue: pg0 loads per-point + squares on the fast engines
            # (DVE/ACT only -- Pool's 4.3us ops would gate the first stats),
            # then pg0's stats + batched smalls; wB slice 0 follows pg0's
            # data in the DMA queue; pg1's first chunk last
            chunk_a(0, split=True)
            chunk_b(0, split=True)
            for j, eng in ((3, "a"), (4, "v"), (0, "a"), (1, "v"), (2, "a")):
                square_point(0, j, eng)
            wB = consts.tile([128, P, C], bf16)

            def load_wb(pg):
                nc.sync.dma_start(
                    out=wB[:, pg * PG:(pg + 1) * PG, :],
                    in_=bass.AP(wb, pg * PG * C, [[0, 128], [1, PG * C]]))

            load_wb(0)
            nc.sync.dma_start(
                out=sbB[:],
                in_=bass.AP(sbb, 0, [[0, 128], [0, NBT], [2 * PG, NPG],
                                     [2, PG], [1, 2]]))
            nc.sync.dma_start(out=lwT[:], in_=lwt[:])
            nc.sync.dma_start(out=bT5[:], in_=bt5[:])
            for j in (3, 4, 0, 1, 2):
                stats_point(0, j)
            cbatch(0)
            # keep PE continuously busy through the first real transposes
            # (the p-state ramp resets on idle)
            for _ in range(20):
                nc.tensor.matmul(wsc[:, 80:208], i16[:], i16[:],
                                 start=True, stop=True,
                                 skip_group_check=True)
            chunk_a(1)

            n = len(iters)
            for k in range(n + 3):
                if k < n:
                    stageA(k)
                # E before D: relu/ost(k-3) must not queue behind later
                # work on the in-order ACT queue
                if 0 <= k - 3 < n:
                    stageE(k - 3)
                if 0 <= k - 2 < n:
                    stageD(k - 2)
                if k < n:
                    pg, ibt = iters[k]
                    if ibt in SQ_SLOT:
                        square_point(pg + 1, SQ_SLOT[ibt])
                    if ibt in ST_SLOT:
                        stats_point(pg + 1, ST_SLOT[ibt])
                    if ibt == 7:
                        cbatch(pg + 1)

    nc.compile()
    return nc


def _get_nc(has_gamma, has_beta, has_linb):
    key = (has_gamma, has_beta, has_linb)
    if key not in _cache:
        _cache[key] = _build(*key)
    return _cache[key]


def _prep(features, weights, bias, ln_gamma, ln_beta, lin_w, lin_b):
    """Host-side layout prep + tiny parameter folds (no bulk math on f)."""
    import ml_dtypes
    bf = ml_dtypes.bfloat16

    f32 = np.float32
    features = np.asarray(features, f32)
    weights = np.asarray(weights, f32)
    bias = np.asarray(bias, f32)
    lin_w = np.asarray(lin_w, f32)

    wrs = weights.sum(-1)                       # [P, C]
    sb = bias.sum(-1) / C                       # [P]
    sb2 = np.square(bias).sum(-1) / C           # [P]

    # x: [P, BS-per-core, C] bf16; xt: [P, C, BS-per-core] fp8 (stats only)
    f8 = ml_dtypes.float8_e4m3
    xp = np.ascontiguousarray(features.transpose(1, 0, 2)).astype(bf)
    xtp = np.ascontiguousarray(features.transpose(1, 2, 0)).astype(f8)

    wrs_h = wrs.reshape(P, 2, 128)              # [pt, h, c]
    bias_h = bias.reshape(P, 2, 128)

    bt5 = np.zeros((5, P, 2, 128), f32)
    for pt in range(P):
        bt5[pt % PG, pt] = bias_h[pt]

    # fp8 stats weights are UNSCALED (wrs/C would be subnormal in e4m3);
    # the 1/C is applied in the on-chip stat read-out instead
    wmv = np.zeros((128, P, 2, 2), f32)
    wmv[:, :, :, 0] = wrs_h.transpose(2, 0, 1)
    wmv[:, :, :, 1] = 2.0 * (wrs_h * bias_h).transpose(2, 0, 1)
    wmv2 = np.square(wrs_h).transpose(2, 0, 1)       # [c, pt, h]

    sbb = np.stack([sb, sb2], axis=-1).reshape(-1)   # [P*2]
    lwt = lin_w.reshape(D, 2, 128).transpose(2, 1, 0)  # [c, h, d]

    common = {
        "wb": wrs.reshape(-1).astype(bf),
        "bt5": bt5.astype(bf),
        "wmv": wmv.astype(f8),
        "wmv2": np.ascontiguousarray(wmv2).astype(f8),
        "sbb": sbb.astype(f32),
        "lwt": np.ascontiguousarray(lwt).astype(bf),
        "g": np.asarray(ln_gamma, f32),
        "bt": np.asarray(ln_beta, f32),
        "lb": np.asarray(lin_b, f32),
    }
    in_maps = []
    for i in range(NCORES):
        m = dict(common)
        m["x"] = np.ascontiguousarray(xp[:, i * BS:(i + 1) * BS, :])
        m["xt"] = np.ascontiguousarray(xtp[:, :, i * BS:(i + 1) * BS])
        in_maps.append(m)
    return in_maps


def kernel(features, weights, bias, ln_gamma, ln_beta, lin_w, lin_b):
    from concourse.bass_utils import run_bass_kernel_spmd

    ln_gamma = np.asarray(ln_gamma, np.float32)
    ln_beta = np.asarray(ln_beta, np.float32)
    lin_b = np.asarray(lin_b, np.float32)
    has_gamma = not np.all(ln_gamma == 1.0)
    has_beta = not np.all(ln_beta == 0.0)
    has_linb = not np.all(lin_b == 0.0)
    nc = _get_nc(has_gamma, has_beta, has_linb)

    in_maps = _prep(features, weights, bias, ln_gamma, ln_beta, lin_w, lin_b)
    res = run_bass_kernel_spmd(nc, in_maps, core_ids=list(range(NCORES)))
    out = np.empty((B, P, D), dtype=np.float32)
    for i in range(NCORES):
        out[i * BS:(i + 1) * BS] = res.results[i]["y"]
    return out


# revision 91
# speedup vs baseline: 1.7105x; 1.0006x over previous
"""Trainium2 Bass kernel for nn_DiseaseHead.

Computation (per the reference):
    w_rs = weights.sum(-1)                       # [P, C]
    pw   = features * w_rs + bias                # [B, P, C]
    pwn  = LayerNorm_c(pw) * gamma + beta
    h    = relu(features + pwn)
    out  = einsum("bpc,dc->bpd", h, lin_w) + lin_b

Sharding: data-parallel over batch B across 8 NeuronCores; parameters
replicated.  Main datapath bf16; the LN-stats side band runs on an fp8
copy of the features (tolerance 2e-2).

Structure (per core: BS=1024 rows, 25 points in 5 groups of PG=5):
 - x loaded [128b, 5p, 256c] bf16 per iteration (pg, b-chunk); xt loaded
   fp8 [c, b] strips per point-group, prefetched ~1.5 groups ahead.
 - LN stats run a FULL GROUP AHEAD of the main pipeline: fT / fT^2 as
   PE stationary, unscaled fp8 weight columns moving; all 8 b-chunks of
   a group accumulate into one PSUM tile.  One batched read-out applies
   1/C + the bias constants, then var/rsqrt/m2 on [128, 40] tiles and a
   single [40, 128] transpose of r for the rank-matmul side.
 - per iteration: pw1 = f*w_rs (DVE tt), q = pw1*r + (-r*mu) (DVE 4x
   tensor_scalar), zT = T(q) + T(f) + bias^T (x) r in PSUM (identity /
   rrow moving), ACT drains with fused Relu, proj against lin_w^T, out
   rows accumulate in SBUF and store once per b-chunk.
"""

import sys

if "/opt/trn_rl_repo" not in sys.path:
    sys.path.insert(0, "/opt/trn_rl_repo")

import numpy as np

B, P, C, D = 8192, 25, 256, 5
NCORES = 8
BS = B // NCORES          # 1024 batch rows per core
PG = 5                    # points per group
NPG = P // PG             # 5 point-groups
NBT = BS // 128           # 8 batch tiles of 128 rows
LN_EPS = 1e-5

_cache = {}

# schedule knobs (sweepable): iteration slots for prefetch triggers
TRIG_A = 2      # ibt at which chunk_a(pg+2) is issued
TRIG_B = 6      # ibt at which chunk_b(pg+2) is issued
TRIG_B1 = 1     # ibt (pg=0) at which chunk_b(1) is issued
# f^2 engine per point and issue slots (for pg+1, during pg); stats
# bursts are timed one slot after their square completes so the waiting
# matmuls never head-block the in-order PE queue
SQ_ENG = {0: "v", 1: "p", 2: "a", 3: "p", 4: "p"}
SQ_SLOT = {0: 3, 1: 4, 2: 2, 3: 0, 4: 1}
ST_SLOT = {3: 3, 4: 2, 5: 4, 6: 0, 7: 1}


def _build(has_gamma: bool, has_beta: bool, has_linb: bool):
    import concourse.bass as bass
    import concourse.tile as tile
    from concourse import bacc, mybir
    from concourse.masks import make_identity

    f32 = mybir.dt.float32
    bf16 = mybir.dt.bfloat16
    f8 = mybir.dt.float8e4
    Alu = mybir.AluOpType
    Act = mybir.ActivationFunctionType

    nc = bacc.Bacc("TRN2", target_bir_lowering=False, debug=False,
                   num_devices=NCORES)

    x = nc.dram_tensor("x", [P, BS, C], bf16, kind="ExternalInput")
    xt = nc.dram_tensor("xt", [P, C, BS], f8, kind="ExternalInput")
    wb = nc.dram_tensor("wb", [P * C], bf16, kind="ExternalInput")
    bt5 = nc.dram_tensor("bt5", [5, P, 2, 128], bf16, kind="ExternalInput")
    wmv = nc.dram_tensor("wmv", [128, P, 2, 2], f8, kind="ExternalInput")
    wmv2 = nc.dram_tensor("wmv2", [128, P, 2], f8, kind="ExternalInput")
    sbb = nc.dram_tensor("sbb", [P * 2], f32, kind="ExternalInput")
    lwt = nc.dram_tensor("lwt", [128, 2, D], bf16, kind="ExternalInput")
    g = nc.dram_tensor("g", [C], f32, kind="ExternalInput")
    bt_ = nc.dram_tensor("bt", [C], f32, kind="ExternalInput")
    lb = nc.dram_tensor("lb", [D], f32, kind="ExternalInput")
    y = nc.dram_tensor("y", [BS, P, D], f32, kind="ExternalOutput")

    with tile.TileContext(nc) as tc:
        with (
            tc.tile_pool(name="consts", bufs=1) as consts,
            tc.tile_pool(name="ftp", bufs=3) as ftp,
            tc.tile_pool(name="f2tp", bufs=3) as f2tp,
            tc.tile_pool(name="fp", bufs=8) as fp,
            tc.tile_pool(name="pw1p", bufs=4) as pw1p,
            tc.tile_pool(name="qp", bufs=4) as qp,
            tc.tile_pool(name="htp", bufs=3) as htp,
            tc.tile_pool(name="statp", bufs=2) as statp,
            tc.tile_pool(name="smallp", bufs=8) as smallp,
            tc.tile_pool(name="rrowp", bufs=4) as rrowp,
            tc.tile_pool(name="zt", bufs=2, space="PSUM") as ztp,
            tc.tile_pool(name="smps", bufs=2, space="PSUM") as smps,
        ):
            ident = consts.tile([128, 128], f32)
            make_identity(nc, ident[:])
            i16 = consts.tile([128, 128], bf16)
            nc.vector.tensor_copy(i16[:], ident[:])

            eps_t = consts.tile([128, 1], f32)
            nc.vector.memset(eps_t[:], LN_EPS)

            bT5 = consts.tile([5, P, 2, 128], bf16)
            wMov = consts.tile([128, P, 2, 2], f8)
            wMov2 = consts.tile([128, P, 2], f8)
            # sb/sb2 constants broadcast over partitions AND b-chunks
            # (loaded after pg0's data -- first needed at cbatch(0))
            sbB = consts.tile([128, NBT, NPG, PG, 2], f32)
            lwT = consts.tile([128, 2, D], bf16)

            if has_gamma:
                gB = consts.tile([128, C], bf16)
                nc.gpsimd.dma_start(
                    out=gB[:], in_=bass.AP(g, 0, [[0, 128], [1, C]]))
            if has_beta:
                bB2 = consts.tile([128, C], bf16)
                nc.gpsimd.dma_start(
                    out=bB2[:], in_=bass.AP(bt_, 0, [[0, 128], [1, C]]))
            if has_linb:
                lbB = consts.tile([128, PG, D], f32)
                nc.gpsimd.dma_start(
                    out=lbB[:], in_=bass.AP(lb, 0, [[0, 128], [0, PG], [1, D]]))

            # per-b-chunk output accumulator: filled across the 5 point
            # groups, stored once per b-chunk (8 big stores vs 40 small)
            yacc = consts.tile([128, NBT, NPG, PG, D], f32)

            pg_tiles = {}
            pg_smt = {}
            pg_stats = {}

            def _xt_dma(ft, pg, a, b):
                nc.sync.dma_start(
                    out=ft[:, a:b],
                    in_=bass.AP(xt, (pg * PG + a) * C * BS,
                                [[BS, 128], [C * BS, b - a],
                                 [128 * BS, 2], [1, 1024]]))

            def chunk_a(pg, split=False):
                """Allocate pg's tiles and load points 3:5 (whose squaring
                slots come first).  Chunked so no single transfer
                head-blocks the per-iter f loads on the DMA engine pool."""
                if pg in pg_tiles or pg >= NPG:
                    return
                ft = ftp.tile([128, PG, 2, 1024], f8, tag="ft")
                for a, b in ((3, 4), (4, PG)) if split else ((3, PG),):
                    _xt_dma(ft, pg, a, b)
                f2t = f2tp.tile([128, PG, 2, 1024], f8, tag="f2t")
                pg_tiles[pg] = (ft, f2t)
                if pg - 3 in pg_tiles:
                    del pg_tiles[pg - 3]

            def get_smt(pg):
                """Per-group stats PSUM, allocated lazily at the first
                stats burst: cols 0:80 = [NBT, 2*PG] accum; cols 80:208 =
                per-iteration rT scratch (keeps the r-transpose chain off
                the zt-recycle loop)."""
                if pg not in pg_smt:
                    smt = smps.tile([128, 208], f32, tag="smt")
                    pg_smt[pg] = smt
                    if pg - 2 in pg_smt:
                        del pg_smt[pg - 2]
                return pg_smt[pg]

            def chunk_b(pg, split=False):
                if pg >= NPG:
                    return
                ft = pg_tiles[pg][0]
                for a, b in ((0, 1), (1, 2), (2, 3)) if split else ((0, 3),):
                    _xt_dma(ft, pg, a, b)

            def square_point(pg, j, eng=None):
                """Emit the f^2 op for point j of group pg, spread one per
                iteration and split across DVE/ACT/Pool."""
                if pg >= NPG:
                    return
                ft, f2t = pg_tiles[pg]
                sl = slice(j, j + 1)
                eng = eng or SQ_ENG[j]
                if eng == "v":
                    nc.vector.tensor_tensor(
                        out=f2t[:, sl], in0=ft[:, sl], in1=ft[:, sl],
                        op=Alu.mult)
                elif eng == "a":
                    nc.scalar.activation(f2t[:, sl], ft[:, sl], Act.Square)
                else:
                    nc.gpsimd.tensor_tensor(
                        out=f2t[:, sl], in0=ft[:, sl], in1=ft[:, sl],
                        op=Alu.mult)

            def stats_point(pg, j):
                """Stats matmuls for point j of pg, ALL 8 b-chunks -- runs
                a full group ahead (only needs the prefetched ft/f2t).
                Cols per point: 0 -> mu-sum, 1 -> E2-sum (unscaled)."""
                if pg >= NPG:
                    return
                ft, f2t = pg_tiles[pg]
                smt = get_smt(pg)
                pt = pg * PG + j
                for ibt in range(NBT):
                    bs = slice(ibt * 128, (ibt + 1) * 128)
                    c0 = ibt * 2 * PG + 2 * j
                    nc.tensor.matmul(
                        smt[:, c0:c0 + 2], ft[:, j, 0, bs],
                        wMov[:, pt, 0, :], start=True, stop=False,
                        skip_group_check=True)
                    nc.tensor.matmul(
                        smt[:, c0 + 1:c0 + 2], f2t[:, j, 0, bs],
                        wMov2[:, pt, 0:1], start=False, stop=False,
                        skip_group_check=True)
                    nc.tensor.matmul(
                        smt[:, c0 + 1:c0 + 2], f2t[:, j, 1, bs],
                        wMov2[:, pt, 1:2], start=False, stop=False,
                        skip_group_check=True)
                    nc.tensor.matmul(
                        smt[:, c0:c0 + 2], ft[:, j, 1, bs],
                        wMov[:, pt, 1, :], start=False, stop=True,
                        skip_group_check=True)

            def cbatch(pg):
                """Batched LN smalls for the whole group [128, NBT, PG]:
                one read-out (applies 1/C + sb consts), var, rsqrt, m2,
                and ONE [40, 128] transpose of r."""
                if pg >= NPG:
                    return
                smt = pg_smt[pg]
                stat = statp.tile([128, NBT, PG, 2], f32, tag="stat")
                nc.vector.scalar_tensor_tensor(
                    out=stat[:], in0=smt[:, 0:NBT * 2 * PG], scalar=1.0 / C,
                    in1=sbB[:, :, pg], op0=Alu.mult, op1=Alu.add)
                mu = stat[:, :, :, 0]
                e2 = stat[:, :, :, 1]
                u = smallp.tile([128, NBT, PG], f32, tag="u")
                nc.vector.tensor_tensor(out=u[:], in0=mu, in1=mu, op=Alu.mult)
                v = smallp.tile([128, NBT, PG], f32, tag="v")
                nc.vector.scalar_tensor_tensor(
                    out=v[:], in0=e2, scalar=1.0, in1=u[:],
                    op0=Alu.bypass, op1=Alu.subtract)
                w = smallp.tile([128, NBT, PG], f32, tag="w")
                nc.scalar.activation(w[:], v[:], Act.Sqrt, bias=eps_t[:])
                r32 = smallp.tile([128, NBT, PG], f32, tag="r32")
                nc.vector.reciprocal(r32[:], w[:])
                m2 = smallp.tile([128, NBT, PG], f32, tag="m2")
                nc.vector.scalar_tensor_tensor(
                    out=m2[:], in0=mu, scalar=-1.0, in1=r32[:],
                    op0=Alu.mult, op1=Alu.mult)
                # bf16 copy of r for the cheap per-iter [5,128] transpose
                rb = smallp.tile([128, NBT, PG], bf16, tag="rb")
                nc.vector.tensor_copy(rb[:], r32[:])
                pg_stats[pg] = (r32, m2, rb)
                if pg - 2 in pg_stats:
                    del pg_stats[pg - 2]

            iters = [(pg, ibt) for pg in range(NPG) for ibt in range(NBT)]
            st = {}
            rrows = {}

            def stageA(i):
                pg, ibt = iters[i]
                p0, b0 = pg * PG, ibt * 128
                # the latency-critical per-iter f load goes FIRST; bulk
                # prefetches for later groups queue behind it
                f = fp.tile([128, PG, C], bf16, tag="f")
                nc.sync.dma_start(
                    out=f[:],
                    in_=bass.AP(x, (p0 * BS + b0) * C,
                                [[C, 128], [BS * C, PG], [1, C]]))
                st[i] = {"f": f, "pg": pg, "ibt": ibt}
                if pg == 0 and ibt == TRIG_B1:
                    chunk_b(1)
                if ibt == TRIG_A:
                    chunk_a(pg + 2)
                    if pg + 1 < NPG:
                        load_wb(pg + 1)
                if ibt == TRIG_B:
                    chunk_b(pg + 2)

            def stageD(i):
                s = st[i]
                pg, ibt = s["pg"], s["ibt"]
                p0 = pg * PG
                f = s["f"]
                r32, m2, rb = pg_stats[pg]
                # pw1 = f * w_rs  (bf16 TT on DVE)
                pw1 = pw1p.tile([128, PG, C], bf16, tag="pw1")
                nc.vector.tensor_tensor(
                    out=pw1[:], in0=f[:], in1=wB[:, p0:p0 + PG, :],
                    op=Alu.mult)
                # q = pw1 * r + (-r*mu)   per point (DVE 4x tensor_scalar)
                q = qp.tile([128, PG, C], bf16, tag="q")
                for ip in range(PG):
                    nc.vector.tensor_scalar(
                        out=q[:, ip, :], in0=pw1[:, ip, :],
                        scalar1=r32[:, ibt, ip:ip + 1],
                        scalar2=m2[:, ibt, ip:ip + 1],
                        op0=Alu.mult, op1=Alu.add)
                if has_gamma:
                    for ip in range(PG):
                        nc.vector.tensor_tensor(
                            out=q[:, ip, :], in0=q[:, ip, :], in1=gB[:],
                            op=Alu.mult)
                if has_beta:
                    for ip in range(PG):
                        nc.vector.tensor_tensor(
                            out=q[:, ip, :], in0=q[:, ip, :], in1=bB2[:],
                            op=Alu.add)
                # zT[(pt,h)] = T(q) + T(f) + bias^T (x) r   (f32 PSUM accum)
                # extra 6th "point" slot: proj output + rT scratch (the
                # pool rounds to 3 banks anyway; same D->E lifetime)
                zt = ztp.tile([128, PG + 1, 2, 128], f32, tag="zt")
                rT = pg_smt[pg][0:PG, 80:208]
                nc.tensor.matmul(rT, rb[:, ibt, :], i16[:],
                                 start=True, stop=True,
                                 skip_group_check=True)
                rrow = rrowp.tile([PG, 128], bf16, tag="rrow")
                nc.vector.tensor_copy(rrow[:], rT)
                for ip in range(PG):
                    pt = p0 + ip
                    for h in range(2):
                        nc.tensor.matmul(
                            zt[:, ip, h, :],
                            q[:, ip, h * 128:(h + 1) * 128], i16[:],
                            start=True, stop=False, skip_group_check=True)
                        nc.tensor.matmul(
                            zt[:, ip, h, :],
                            f[:, ip, h * 128:(h + 1) * 128], i16[:],
                            start=False, stop=False, skip_group_check=True)
                        nc.tensor.matmul(
                            zt[:, ip, h, :], bT5[0:PG, pt, h, :], rrow[:],
                            start=False, stop=True, skip_group_check=True)
                s["zt"] = zt

            def stageE(i):
                s = st[i]
                pg, ibt = s["pg"], s["ibt"]
                b0 = ibt * 128
                zt = s["zt"]
                # ACT drain with fused Relu -> hT bf16
                ht = htp.tile([128, PG, 2, 128], bf16, tag="ht")
                nc.scalar.activation(ht[:], zt[:, 0:PG, :, :], Act.Relu)
                # proj: stationary hT slice, moving lwT half
                pr = zt[:, PG, 0, 0:PG * D]
                for ip in range(PG):
                    for h in range(2):
                        nc.tensor.matmul(
                            pr[:, ip * D:(ip + 1) * D],
                            ht[:, ip, h, :], lwT[:, h, :],
                            start=(h == 0), stop=(h == 1),
                            skip_group_check=True)
                ost = yacc[:, ibt, pg]
                if has_linb:
                    nc.vector.tensor_tensor(
                        out=ost, in0=pr,
                        in1=lbB[:, 0:PG, :], op=Alu.add)
                else:
                    nc.scalar.copy(ost, pr)
                if pg == NPG - 1:
                    nc.gpsimd.dma_start(
                        out=bass.AP(y, b0 * P * D,
                                    [[P * D, 128], [1, P * D]]),
                        in_=yacc[:, ibt])
                del st[i]

            # PE p-state warmup: ~3us of dependency-free matmuls so the
            # first real transposes run at full clock (scratch: rT region)
            wsc = smps.tile([128, 208], f32, tag="smt")
            for _ in range(48):
                nc.tensor.matmul(wsc[:, 80:208], i16[:], i16[:],
                                 start=True, stop=True,
                                 skip_group_check=True)

            # prologue: pg0 loads per-point + squares on the fast engines
            # (DVE/ACT only -- Pool's 4.3us ops would gate the first stats),
            # then pg0's stats + batched smalls; wB slice 0 follows pg0's
            # data in the DMA queue; pg1's first chunk last
            chunk_a(0, split=True)
            chunk_b(0, split=True)
            nc.sync.dma_start(out=wMov[:], in_=wmv[:])
            nc.sync.dma_start(out=wMov2[:], in_=wmv2[:])
            # first-arriving point on Pool (it runs while ACT/DVE chew
            # later arrivals), then alternate ACT/DVE by arrival order
            for j, eng in ((3, "p"), (4, "a"), (0, "v"), (1, "a"), (2, "v")):
                square_point(0, j, eng)
            wB = consts.tile([128, P, C], bf16)

            def load_wb(pg):
                nc.sync.dma_start(
                    out=wB[:, pg * PG:(pg + 1) * PG, :],
                    in_=bass.AP(wb, pg * PG * C, [[0, 128], [1, PG * C]]))

            load_wb(0)
            nc.sync.dma_start(
                out=sbB[:],
                in_=bass.AP(sbb, 0, [[0, 128], [0, NBT], [2 * PG, NPG],
                                     [2, PG], [1, 2]]))
            nc.sync.dma_start(out=lwT[:], in_=lwt[:])
            nc.sync.dma_start(out=bT5[:], in_=bt5[:])
            for j in (3, 4, 0, 1, 2):
                stats_point(0, j)
            cbatch(0)
            # keep PE continuously busy through the first real transposes
            # (the p-state ramp resets on idle)
            for _ in range(20):
                nc.tensor.matmul(wsc[:, 80:208], i16[:], i16[:],
                                 start=True, stop=True,
                                 skip_group_check=True)
            chunk_a(1)

            n = len(iters)
            for k in range(n + 3):
                if k < n:
                    stageA(k)
                # E before D: relu/ost(k-3) must not queue behind later
                # work on the in-order ACT queue
                if 0 <= k - 3 < n:
                    stageE(k - 3)
                if 0 <= k - 2 < n:
                    stageD(k - 2)
                if k < n:
                    pg, ibt = iters[k]
                    if ibt in SQ_SLOT:
                        square_point(pg + 1, SQ_SLOT[ibt])
                    if ibt in ST_SLOT:
                        stats_point(pg + 1, ST_SLOT[ibt])
                    if ibt == 7:
                        cbatch(pg + 1)

    nc.compile()
    return nc


def _get_nc(has_gamma, has_beta, has_linb):
    key = (has_gamma, has_beta, has_linb)
    if key not in _cache:
        _cache[key] = _build(*key)
    return _cache[key]


def _prep(features, weights, bias, ln_gamma, ln_beta, lin_w, lin_b):
    """Host-side layout prep + tiny parameter folds (no bulk math on f)."""
    import ml_dtypes
    bf = ml_dtypes.bfloat16

    f32 = np.float32
    features = np.asarray(features, f32)
    weights = np.asarray(weights, f32)
    bias = np.asarray(bias, f32)
    lin_w = np.asarray(lin_w, f32)

    wrs = weights.sum(-1)                       # [P, C]
    sb = bias.sum(-1) / C                       # [P]
    sb2 = np.square(bias).sum(-1) / C           # [P]

    # x: [P, BS-per-core, C] bf16; xt: [P, C, BS-per-core] fp8 (stats only)
    f8 = ml_dtypes.float8_e4m3
    xp = np.ascontiguousarray(features.transpose(1, 0, 2)).astype(bf)
    xtp = np.ascontiguousarray(features.transpose(1, 2, 0)).astype(f8)

    wrs_h = wrs.reshape(P, 2, 128)              # [pt, h, c]
    bias_h = bias.reshape(P, 2, 128)

    bt5 = np.zeros((5, P, 2, 128), f32)
    for pt in range(P):
        bt5[pt % PG, pt] = bias_h[pt]

    # fp8 stats weights are UNSCALED (wrs/C would be subnormal in e4m3);
    # the 1/C is applied in the on-chip stat read-out instead
    wmv = np.zeros((128, P, 2, 2), f32)
    wmv[:, :, :, 0] = wrs_h.transpose(2, 0, 1)
    wmv[:, :, :, 1] = 2.0 * (wrs_h * bias_h).transpose(2, 0, 1)
    wmv2 = np.square(wrs_h).transpose(2, 0, 1)       # [c, pt, h]

    sbb = np.stack([sb, sb2], axis=-1).reshape(-1)   # [P*2]
    lwt = lin_w.reshape(D, 2, 128).transpose(2, 1, 0)  # [c, h, d]

    common = {
        "wb": wrs.reshape(-1).astype(bf),
        "bt5": bt5.astype(bf),
        "wmv": wmv.astype(f8),
        "wmv2": np.ascontiguousarray(wmv2).astype(f8),
        "sbb": sbb.astype(f32),
        "lwt": np.ascontiguousarray(lwt).astype(bf),
        "g": np.asarray(ln_gamma, f32),
        "bt": np.asarray(ln_beta, f32),
        "lb": np.asarray(lin_b, f32),
    }
    in_maps = []
    for i in range(NCORES):
        m = dict(common)
        m["x"] = np.ascontiguousarray(xp[:, i * BS:(i + 1) * BS, :])
        m["xt"] = np.ascontiguousarray(xtp[:, :, i * BS:(i + 1) * BS])
        in_maps.append(m)
    return in_maps


def kernel(features, weights, bias, ln_gamma, ln_beta, lin_w, lin_b):
    from concourse.bass_utils import run_bass_kernel_spmd

    ln_gamma = np.asarray(ln_gamma, np.float32)
    ln_beta = np.asarray(ln_beta, np.float32)
    lin_b = np.asarray(lin_b, np.float32)
    has_gamma = not np.all(ln_gamma == 1.0)
    has_beta = not np.all(ln_beta == 0.0)
    has_linb = not np.all(lin_b == 0.0)
    nc = _get_nc(has_gamma, has_beta, has_linb)

    in_maps = _prep(features, weights, bias, ln_gamma, ln_beta, lin_w, lin_b)
    res = run_bass_kernel_spmd(nc, in_maps, core_ids=list(range(NCORES)))
    out = np.empty((B, P, D), dtype=np.float32)
    for i in range(NCORES):
        out[i * BS:(i + 1) * BS] = res.results[i]["y"]
    return out
